# revision 1
# baseline (speedup 1.0000x reference)
"""Trainium2 Bass kernel v2: 16-head RoPE attention (B=2, L=2048, HIDDEN=1024).

Sharding: 8 cores = 2 batches x 4 head-groups (4 heads per core).

v2 redesign vs baseline:
- Single pass over x (kept resident in SBUF, bf16), one fetch instead of two.
- bf16 intermediates everywhere (q_ro/k_ro/v/ex/o_sb) for 2-4x DVE rate and
  half the DMA bytes; PSUM stays fp32.
- PV stationary per head is [V|1] (head0) / [1|V] (head1): softmax sums come
  out replicated across 64 psum partitions, so normalization is just
  reciprocal (partition-shifting) + tensor_mul on DVE. No partition
  broadcast, no gpsimd round trips.
- One long attention window: exp (ACT engine) is the per-kt bottleneck
  (~1.05us vs 0.85us PE), so remaining projection and o_proj matmuls are
  interleaved into attention's PE slack to keep PE ~100% busy and dodge
  p-state downclocking.
"""

import numpy as np
from contextlib import ExitStack

from concourse import bacc, tile, mybir
from concourse.bass import ts
from concourse.bass_utils import run_bass_kernel_spmd

HIDDEN = 1024
HEADS = 16
HD = 64
L = 2048
B = 2
BASE = 10000.0

P = 128
E_LOCAL = 256          # 4 heads per core
N_PAIRS = 2            # head pairs per core (2 heads across 128 partitions)
HC = HIDDEN // P       # 8 hidden chunks
TC = 512               # token chunk (matmul free dim)
N_TC = L // TC         # 4
N_TT = L // P          # 16 token tiles (for v / k-tiles)
SCALE = 1.0 / 8.0      # 1/sqrt(HD)

F32 = mybir.dt.float32
F32R = mybir.dt.float32r
BF16 = mybir.dt.bfloat16
AF = mybir.ActivationFunctionType


def build_program(debug=False):
    nc = bacc.Bacc(None, target_bir_lowering=False)
    names = {}
    with tile.TileContext(nc) as tc:
        ctx = ExitStack()
        with ctx:
            dram = ctx.enter_context(tc.tile_pool(name="dram", bufs=1, space="DRAM"))
            xT_d = dram.tile([P, N_TC, HC, TC], BF16, kind="ExternalInput", name="xT")
            wq_d = dram.tile([P, HC, E_LOCAL], BF16, kind="ExternalInput", name="wq")
            wk_d = dram.tile([P, HC, E_LOCAL], BF16, kind="ExternalInput", name="wk")
            wv_d = dram.tile([P, HC, E_LOCAL], BF16, kind="ExternalInput", name="wv")
            wo_d = dram.tile([P, N_PAIRS, HIDDEN], BF16, kind="ExternalInput", name="wo")
            cos_d = dram.tile([P, L], BF16, kind="ExternalInput", name="cosT")
            sin_d = dram.tile([P, L], BF16, kind="ExternalInput", name="sinT")
            out_d = dram.tile([HIDDEN, L], BF16, kind="ExternalOutput", name="outT")
            names["in"] = [t.tensor.name for t in (xT_d, wq_d, wk_d, wv_d, wo_d, cos_d, sin_d)]
            names["out"] = out_d.tensor.name
            if debug:
                dbg_q = dram.tile([P, 2, L], F32, kind="ExternalOutput", name="dbg_q")
                dbg_k = dram.tile([P, 2, L], F32, kind="ExternalOutput", name="dbg_k")
                dbg_v = dram.tile([P, N_TT, 2, 2, P], F32, kind="ExternalOutput", name="dbg_v")
                dbg_o = dram.tile([P, 2, L], F32, kind="ExternalOutput", name="dbg_o")
                dbg_ch = dram.tile([P, 5, TC], F32, kind="ExternalOutput", name="dbg_ch")
                names["dbg"] = [t.tensor.name for t in (dbg_q, dbg_k, dbg_v, dbg_o, dbg_ch)]

            # ---------------- persistent SBUF ----------------
            const = ctx.enter_context(tc.tile_pool(name="const", bufs=1))
            x_sb = const.tile([P, N_TC, HC, TC], BF16)     # 32 KB/part, t-major
            wq_sb = const.tile([P, HC, E_LOCAL], BF16)
            wk_sb = const.tile([P, HC, E_LOCAL], BF16)
            wv_sb = const.tile([P, HC, E_LOCAL], BF16)
            wo_sb = const.tile([P, N_PAIRS, HIDDEN], BF16)
            cos_sb = const.tile([P, L], BF16)
            sin_sb = const.tile([P, L], BF16)
            q_ro = [const.tile([P, L], BF16, name=f"q_ro{p}") for p in range(N_PAIRS)]
            k_ro = [const.tile([P, L], BF16, name=f"k_ro{p}") for p in range(N_PAIRS)]
            # v stationaries: per (tt, pair): slot0=[V_h0|1], slot1=[1|V_h1]
            v_all = const.tile([P, N_TT, N_PAIRS, 2, P], F32R)   # 32 KB/part
            o_sb = [const.tile([P, L], BF16, name=f"o_sb{p}") for p in range(N_PAIRS)]

            # ones: middle 128 cols of each (tt, pair) 256-col block
            ones_sb = const.tile([P, 2 * HD], F32)
            nc.vector.memset(ones_sb[:], 1.0)
            ones3 = ones_sb[:].rearrange("p (s c) -> p s c", s=2)
            for tt in range(N_TT):
                for pr in range(N_PAIRS):
                    # ones columns HD:P of both slots
                    nc.vector.tensor_copy(v_all[:, tt, pr, :, HD:P], ones3)

            # ---------------- working pools ----------------
            rope_t = ctx.enter_context(tc.tile_pool(name="rope", bufs=2))
            expp = ctx.enter_context(tc.tile_pool(name="expp", bufs=5))
            nrm = ctx.enter_context(tc.tile_pool(name="nrm", bufs=2))
            outst = ctx.enter_context(tc.tile_pool(name="outst", bufs=4))

            # ---------------- DMA loads (sync + scalar HWDGE rings) -------
            def load_inputs():
                # x chunks stream on the sync ring; weights/tables go on the
                # scalar ring so the two don't serialize behind each other.
                # All host buffers are pre-arranged so every partition row is
                # one contiguous multi-KB descriptor.
                for t in (0, 1):
                    nc.sync.dma_start(x_sb[:, t, 0:4, :], xT_d[:, t, 0:4, :])
                    nc.sync.dma_start(x_sb[:, t, 4:8, :], xT_d[:, t, 4:8, :])
                for t in (2, 3):
                    nc.sync.dma_start(x_sb[:, t, :, :], xT_d[:, t, :, :])
                nc.scalar.dma_start(wk_sb[:], wk_d[:])
                nc.scalar.dma_start(wv_sb[:], wv_d[:])
                nc.scalar.dma_start(wq_sb[:], wq_d[:])
                nc.scalar.dma_start(cos_sb[:], cos_d[:])
                nc.scalar.dma_start(sin_sb[:], sin_d[:])
                nc.scalar.dma_start(wo_sb[:], wo_d[:])

            # ---------------- building blocks ----------------
            def rope_chunk(dst, ps_tile, t, dbg_cap=False):
                """psum [128, TC] -> dst[:, t*TC:(t+1)*TC] bf16 with RoPE."""
                raw = rope_t.tile([P, TC], BF16, name="raw")
                shuf = rope_t.tile([P, TC], BF16, name="shuf")
                t1 = rope_t.tile([P, TC], BF16, name="t1")
                t2 = rope_t.tile([P, TC], BF16, name="t2")
                nc.vector.tensor_copy(raw[:], ps_tile[:])  # f32 psum -> bf16
                # swap 32-partition halves within each 64-row head block
                for a, b in ((0, 32), (32, 0), (64, 96), (96, 64)):
                    nc.gpsimd.dma_start(shuf[a : a + 32, :], raw[b : b + 32, :])
                cs = cos_sb[:, ts(t, TC)]
                sn = sin_sb[:, ts(t, TC)]
                nc.vector.tensor_mul(t1[:], raw[:], cs)
                nc.vector.tensor_mul(t2[:], shuf[:], sn)
                nc.vector.tensor_add(dst[:, ts(t, TC)], t1[:], t2[:])
                if dbg_cap:
                    nc.gpsimd.dma_start(dbg_ch[:, 1, :], raw[:])
                    nc.gpsimd.dma_start(dbg_ch[:, 2, :], shuf[:])
                    nc.gpsimd.dma_start(dbg_ch[:, 3, :], t1[:])
                    nc.gpsimd.dma_start(dbg_ch[:, 4, :], t2[:])

            def qk_proj_chunk(w_sb, pair, t, dst, ps_qk, dbg_cap=False):
                """One [128, TC] projection chunk + rope. 8 PE matmuls."""
                qp = ps_qk.tile([P, TC], F32, name="qp", tag="qp", bufs=1)
                for h in range(HC):
                    nc.tensor.matmul(
                        qp[:], w_sb[:, h, ts(pair, P)], x_sb[:, t, h, :],
                        start=(h == 0), stop=(h == HC - 1),
                    )
                rope_chunk(dst, qp, t, dbg_cap=dbg_cap)

            def v_chunk(tt, ps_qk):
                """One [128 tok, 256] v tile -> v_all slots. 8 PE matmuls."""
                vp = ps_qk.tile([P, TC], F32, name="vp", tag="kp", bufs=1)
                for h in range(HC):
                    nc.tensor.matmul(
                        vp[:, 0:E_LOCAL], x_sb[:, tt // 4, h, ts(tt % 4, P)], wv_sb[:, h, :],
                        start=(h == 0), stop=(h == HC - 1),
                    )
                for pr in range(N_PAIRS):
                    # slot0 = [V_h0 | 1], slot1 = [V_h1 | 1]
                    nc.vector.tensor_copy(
                        v_all[:, tt, pr, :, 0:HD],
                        vp[:, ts(pr, P)].rearrange("p (s c) -> p s c", c=HD),
                    )

            def o_proj_mm(c, fc, ps_qk, tail=False):
                """One [128, TC] output-projection chunk. 2 PE matmuls + store."""
                op = ps_qk.tile(
                    [P, TC], F32, name="op", tag=("qp" if fc % 2 == 0 else "kp"), bufs=1
                )
                for pair in range(N_PAIRS):
                    nc.tensor.matmul(
                        op[:], wo_sb[:, pair, ts(fc, P)], o_sb[pair][:, ts(c, TC)],
                        start=(pair == 0), stop=(pair == N_PAIRS - 1),
                    )
                ob = outst.tile([P, TC], BF16, name="ob")
                if tail and fc % 2 == 1:
                    nc.scalar.copy(ob[:], op[:])   # ACT is idle after last exp
                else:
                    nc.vector.tensor_copy(ob[:], op[:])
                nc.sync.dma_start(out_d[ts(fc, P), ts(c, TC)], ob[:])

            # ---------------- attention pipeline pieces ----------------
            def scores_exp(pair, c, kt, ps_st):
                st = ps_st.tile([P, 2 * TC], F32, name="st", tag="st")
                nc.tensor.matmul(
                    st[:, 0:TC],
                    k_ro[pair][0:HD, ts(kt, P)],
                    q_ro[pair][0:HD, ts(c, TC)],
                    start=True, stop=True,
                )
                nc.tensor.matmul(
                    st[:, TC : 2 * TC],
                    k_ro[pair][HD:P, ts(kt, P)],
                    q_ro[pair][HD:P, ts(c, TC)],
                    start=True, stop=True,
                    tile_position=(64, 0),
                )
                ex = expp.tile([P, 2 * TC], F32R, name="ex")
                nc.scalar.activation(ex[:], st[:], AF.Exp, scale=SCALE)
                return ex

            def pv_step(pair, c, kt, ex, ots):
                nc.tensor.matmul(
                    ots[0][:], v_all[:, kt, pair, 0, :], ex[:, 0:TC],
                    start=(kt == 0), stop=(kt == N_TT - 1),
                )
                nc.tensor.matmul(
                    ots[1][:], v_all[:, kt, pair, 1, :], ex[:, TC : 2 * TC],
                    start=(kt == 0), stop=(kt == N_TT - 1),
                )

            def norm_c(pair, c, ots):
                # sums are replicated on the "ones" side (psum rows HD:P).
                # Partition moves only ever go DOWN via tensor_copy (up-shifts
                # are silently broken); head1's result is lifted to partitions
                # HD:P by a DMA like baseline.
                ot0, ot1 = ots
                s0 = nrm.tile([HD, TC], F32, name="s0")
                r0 = nrm.tile([HD, TC], F32, name="r0")
                nc.vector.tensor_copy(s0[:], ot0[HD:P, :])
                nc.vector.reciprocal_approx_fast(r0[:], s0[:])
                nc.vector.tensor_mul(o_sb[pair][0:HD, ts(c, TC)], ot0[0:HD, :], r0[:])
                s1 = nrm.tile([HD, TC], F32, name="s1")
                r1 = nrm.tile([HD, TC], F32, name="r1")
                onrm = nrm.tile([HD, TC], BF16, name="onrm")
                nc.vector.tensor_copy(s1[:], ot1[HD:P, :])
                nc.vector.reciprocal_approx_fast(r1[:], s1[:])
                nc.vector.tensor_mul(onrm[:], ot1[0:HD, :], r1[:])
                nc.gpsimd.dma_start(o_sb[pair][HD:P, ts(c, TC)], onrm[:])

            # ---------------- emission schedule ----------------
            # Flat software pipeline over steps s=(pair,c,kt): scores+exp run
            # AHEAD steps in front of PV so the ACT engine never drains at c
            # boundaries. Projection and o_proj work is interleaved into the
            # PE stream at fixed steps (all emission-order deps respected).
            AHEAD = 4
            steps = [
                (pair, c, kt)
                for pair in range(N_PAIRS)
                for c in range(N_TC)
                for kt in range(N_TT)
            ]
            N_STEPS = len(steps)  # 128

            with tc.tile_pool(name="ps_qk", bufs=1, space="PSUM") as ps_qk:
                with tc.tile_pool(name="ps_st", bufs=2, space="PSUM") as ps_st:
                    with tc.tile_pool(name="ps_ot", bufs=1, space="PSUM") as ps_ot:
                        load_inputs()

                        def _qk(w, pair, t, dst, dbg_cap=False):
                            return lambda: qk_proj_chunk(
                                w, pair, t, dst, ps_qk, dbg_cap=dbg_cap and debug
                            )

                        # interleaved PE work, keyed by pipeline loop index i.
                        # Constraints (unit done before): q0[t] < S(p0,c=t);
                        # k1[t] < S(p1,c0,kt=4t); q1[t] < S(p1,c=t); o_proj(c)
                        # after norm(p1,c) which lands at i = 80+16c+1.
                        il = {
                            4: [_qk(wq_sb, 0, 1, q_ro[0])],
                            12: [_qk(wq_sb, 0, 2, q_ro[0])],
                            20: [_qk(wq_sb, 0, 3, q_ro[0])],
                            28: [_qk(wk_sb, 1, 0, k_ro[1])],
                            36: [_qk(wk_sb, 1, 1, k_ro[1])],
                            44: [_qk(wk_sb, 1, 2, k_ro[1])],
                            50: [_qk(wk_sb, 1, 3, k_ro[1])],
                            56: [_qk(wq_sb, 1, 0, q_ro[1], dbg_cap=True)],
                            62: [_qk(wq_sb, 1, 1, q_ro[1])],
                            70: [_qk(wq_sb, 1, 2, q_ro[1])],
                            78: [_qk(wq_sb, 1, 3, q_ro[1])],
                        }
                        for j, fc in enumerate(range(HC)):      # o_proj(c0)
                            il.setdefault(84 + 2 * j, []).append(
                                lambda fc=fc: o_proj_mm(0, fc, ps_qk)
                            )
                        for j, fc in enumerate(range(HC)):      # o_proj(c1)
                            il.setdefault(100 + 2 * j, []).append(
                                lambda fc=fc: o_proj_mm(1, fc, ps_qk)
                            )
                        for j, fc in enumerate(range(HC)):      # o_proj(c2)
                            il.setdefault(115 + 2 * j, []).append(
                                lambda fc=fc: o_proj_mm(2, fc, ps_qk)
                            )

                        ex_q = {}
                        ots = {}

                        def ensure_ots(pair, c):
                            if (pair, c) not in ots:
                                ots[(pair, c)] = (
                                    ps_ot.tile([P, TC], F32, name="ot0", tag="ot0", bufs=1),
                                    ps_ot.tile([P, TC], F32, name="ot1", tag="ot1", bufs=1),
                                )
                            return ots[(pair, c)]

                        for i in range(N_STEPS + AHEAD):
                            # pre-phase work rides in front of its first use
                            if i == 0:
                                qk_proj_chunk(wk_sb, 0, 0, k_ro[0], ps_qk)
                                qk_proj_chunk(wq_sb, 0, 0, q_ro[0], ps_qk)
                                for tt in range(4):
                                    v_chunk(tt, ps_qk)
                            elif i in (1, 2, 3):
                                t = i
                                qk_proj_chunk(wk_sb, 0, t, k_ro[0], ps_qk)
                                for tt in range(4 * t, 4 * t + 4):
                                    v_chunk(tt, ps_qk)

                            if i < N_STEPS:
                                pair, c, kt = steps[i]
                                ensure_ots(pair, c)
                                ex_q[i] = scores_exp(pair, c, kt, ps_st)
                            j = i - AHEAD
                            if j >= 0:
                                pair, c, kt = steps[j]
                                pv_step(pair, c, kt, ex_q.pop(j), ots[(pair, c)])
                                if kt == N_TT - 1:
                                    norm_c(pair, c, ots.pop((pair, c)))
                            for unit in il.get(i, []):
                                unit()

                        for fc in range(HC):
                            o_proj_mm(N_TC - 1, fc, ps_qk, tail=True)

                        if debug:
                            for pr in range(N_PAIRS):
                                nc.gpsimd.dma_start(dbg_q[:, pr, :], q_ro[pr][:])
                                nc.gpsimd.dma_start(dbg_k[:, pr, :], k_ro[pr][:])
                                nc.gpsimd.dma_start(dbg_o[:, pr, :], o_sb[pr][:])
                            nc.sync.dma_start(dbg_v[:], v_all[:].bitcast(F32))

    nc.compile()
    return nc, names


_CACHE = {}


def _get_program():
    if "prog" not in _CACHE:
        _CACHE["prog"] = build_program()
    return _CACHE["prog"]


def _rope_tables():
    inv_freq = 1.0 / (BASE ** (np.arange(0, HD, 2, dtype=np.float64) / HD))
    t = np.arange(L, dtype=np.float64)
    freqs = np.outer(t, inv_freq)            # [L, 32]
    emb = np.concatenate((freqs, freqs), -1)  # [L, 64]
    cos = np.cos(emb).T.astype(np.float32)    # [64, L]
    sin = np.sin(emb).T.astype(np.float32)    # [64, L]
    sin_signed = sin.copy()
    sin_signed[: HD // 2] *= -1.0             # rotate_half sign baked in
    cosT = np.ascontiguousarray(np.concatenate([cos, cos], 0))      # [128, L]
    sinT = np.ascontiguousarray(np.concatenate([sin_signed, sin_signed], 0))
    return cosT, sinT


def _bf16(a):
    import ml_dtypes
    return np.ascontiguousarray(a).astype(ml_dtypes.bfloat16)


def make_in_maps(names, x, Wq, Wk, Wv, Wo):
    cosT, sinT = _rope_tables()
    cosT, sinT = _bf16(cosT), _bf16(sinT)
    in_maps = []
    # x: [1024, 2048] -> [128 p, 4 t, 8 c, 512] so each partition row is one
    # 8KB-contiguous DMA descriptor. Weights similarly partition-major.
    xTs = [
        _bf16(x[b].T.reshape(8, 128, 4, 512).transpose(1, 2, 0, 3))
        for b in range(B)
    ]
    def _w(a):                     # [c*128, e] -> [128, c, e]
        return _bf16(a.reshape(a.shape[0] // 128, 128, -1).transpose(1, 0, 2))
    for core in range(8):
        b = core // 4
        g = core % 4
        es = slice(g * E_LOCAL, (g + 1) * E_LOCAL)
        m = {
            names["in"][0]: xTs[b],
            names["in"][1]: _w(Wq[es, :].T),
            names["in"][2]: _w(Wk[es, :].T),
            names["in"][3]: _w(Wv[es, :].T),
            names["in"][4]: _w(Wo[:, es].T),   # [256,1024] -> [128, 2, 1024]
            names["in"][5]: cosT,
            names["in"][6]: sinT,
        }
        in_maps.append(m)
    return in_maps


def gather_out(names, res):
    out = np.zeros((B, L, HIDDEN), dtype=np.float32)
    for b in range(B):
        acc = np.zeros((HIDDEN, L), dtype=np.float32)
        for g in range(4):
            acc += np.asarray(res.results[b * 4 + g][names["out"]], dtype=np.float32)
        out[b] = acc.T
    return out


def kernel(x, Wq, Wk, Wv, Wo):
    x = np.asarray(x, dtype=np.float32)
    Wq = np.asarray(Wq, dtype=np.float32)
    Wk = np.asarray(Wk, dtype=np.float32)
    Wv = np.asarray(Wv, dtype=np.float32)
    Wo = np.asarray(Wo, dtype=np.float32)

    nc, names = _get_program()
    in_maps = make_in_maps(names, x, Wq, Wk, Wv, Wo)
    res = run_bass_kernel_spmd(nc, in_maps, core_ids=list(range(8)))
    return gather_out(names, res)



# revision 4
# speedup vs baseline: 1.0669x; 1.0669x over previous
"""Trainium2 Bass kernel v2: 16-head RoPE attention (B=2, L=2048, HIDDEN=1024).

Sharding: 8 cores = 2 batches x 4 head-groups (4 heads per core).

v2 redesign vs baseline:
- Single pass over x (kept resident in SBUF, bf16), one fetch instead of two.
- bf16 intermediates everywhere (q_ro/k_ro/v/ex/o_sb) for 2-4x DVE rate and
  half the DMA bytes; PSUM stays fp32.
- PV stationary per head is [V|1] (head0) / [1|V] (head1): softmax sums come
  out replicated across 64 psum partitions, so normalization is just
  reciprocal (partition-shifting) + tensor_mul on DVE. No partition
  broadcast, no gpsimd round trips.
- One long attention window: exp (ACT engine) is the per-kt bottleneck
  (~1.05us vs 0.85us PE), so remaining projection and o_proj matmuls are
  interleaved into attention's PE slack to keep PE ~100% busy and dodge
  p-state downclocking.
"""

import numpy as np
from contextlib import ExitStack

from concourse import bacc, tile, mybir
from concourse.bass import ts
from concourse.bass_utils import run_bass_kernel_spmd

HIDDEN = 1024
HEADS = 16
HD = 64
L = 2048
B = 2
BASE = 10000.0

P = 128
E_LOCAL = 256          # 4 heads per core
N_PAIRS = 2            # head pairs per core (2 heads across 128 partitions)
HC = HIDDEN // P       # 8 hidden chunks
TC = 512               # token chunk (matmul free dim)
N_TC = L // TC         # 4
N_TT = L // P          # 16 token tiles (for v / k-tiles)
SCALE = 1.0 / 8.0      # 1/sqrt(HD)

F32 = mybir.dt.float32
F32R = mybir.dt.float32r
BF16 = mybir.dt.bfloat16
AF = mybir.ActivationFunctionType


def build_program(debug=False):
    nc = bacc.Bacc(None, target_bir_lowering=False)
    names = {}
    with tile.TileContext(nc) as tc:
        ctx = ExitStack()
        with ctx:
            dram = ctx.enter_context(tc.tile_pool(name="dram", bufs=1, space="DRAM"))
            xT_d = dram.tile([P, N_TC, HC, TC], BF16, kind="ExternalInput", name="xT")
            wq_d = dram.tile([P, HC, E_LOCAL], BF16, kind="ExternalInput", name="wq")
            wk_d = dram.tile([P, HC, E_LOCAL], BF16, kind="ExternalInput", name="wk")
            wv_d = dram.tile([P, HC, E_LOCAL], BF16, kind="ExternalInput", name="wv")
            wo_d = dram.tile([P, N_PAIRS, HIDDEN], BF16, kind="ExternalInput", name="wo")
            cos_d = dram.tile([P, L], BF16, kind="ExternalInput", name="cosT")
            sin_d = dram.tile([P, L], BF16, kind="ExternalInput", name="sinT")
            out_d = dram.tile([HIDDEN, L], BF16, kind="ExternalOutput", name="outT")
            names["in"] = [t.tensor.name for t in (xT_d, wq_d, wk_d, wv_d, wo_d, cos_d, sin_d)]
            names["out"] = out_d.tensor.name
            if debug:
                dbg_q = dram.tile([P, 2, L], F32, kind="ExternalOutput", name="dbg_q")
                dbg_k = dram.tile([P, 2, L], F32, kind="ExternalOutput", name="dbg_k")
                dbg_v = dram.tile([P, N_TT, 2, 2, P], F32, kind="ExternalOutput", name="dbg_v")
                dbg_o = dram.tile([P, 2, L], F32, kind="ExternalOutput", name="dbg_o")
                dbg_ch = dram.tile([P, 5, TC], F32, kind="ExternalOutput", name="dbg_ch")
                names["dbg"] = [t.tensor.name for t in (dbg_q, dbg_k, dbg_v, dbg_o, dbg_ch)]

            # ---------------- persistent SBUF ----------------
            const = ctx.enter_context(tc.tile_pool(name="const", bufs=1))
            x_sb = const.tile([P, N_TC, HC, TC], BF16)     # 32 KB/part, t-major
            wq_sb = const.tile([P, HC, E_LOCAL], BF16)
            wk_sb = const.tile([P, HC, E_LOCAL], BF16)
            wv_sb = const.tile([P, HC, E_LOCAL], BF16)
            wo_sb = const.tile([P, N_PAIRS, HIDDEN], BF16)
            cos_sb = const.tile([P, L], BF16)
            sin_sb = const.tile([P, L], BF16)
            q_ro = [const.tile([P, L], BF16, name=f"q_ro{p}") for p in range(N_PAIRS)]
            k_ro = [const.tile([P, L], BF16, name=f"k_ro{p}") for p in range(N_PAIRS)]
            # v stationaries: per (tt, pair): slot0=[V_h0|1], slot1=[1|V_h1]
            v_all = const.tile([P, N_TT, N_PAIRS, 2, P], BF16)   # 16 KB/part
            o_sb = [const.tile([P, L], BF16, name=f"o_sb{p}") for p in range(N_PAIRS)]

            # ones: middle 128 cols of each (tt, pair) 256-col block
            ones_sb = const.tile([P, 2 * HD], F32)
            nc.vector.memset(ones_sb[:], 1.0)
            ones3 = ones_sb[:].rearrange("p (s c) -> p s c", s=2)
            for tt in range(N_TT):
                for pr in range(N_PAIRS):
                    # ones columns HD:P of both slots
                    nc.vector.tensor_copy(v_all[:, tt, pr, :, HD:P], ones3)

            # ---------------- working pools ----------------
            rope_t = ctx.enter_context(tc.tile_pool(name="rope", bufs=2))
            expp = ctx.enter_context(tc.tile_pool(name="expp", bufs=5))
            nrm = ctx.enter_context(tc.tile_pool(name="nrm", bufs=2))
            outst = ctx.enter_context(tc.tile_pool(name="outst", bufs=4))

            # ---------------- DMA loads (sync + scalar HWDGE rings) -------
            def load_inputs():
                # x chunks stream on the sync ring; weights/tables go on the
                # scalar ring so the two don't serialize behind each other.
                # All host buffers are pre-arranged so every partition row is
                # one contiguous multi-KB descriptor.
                for t in (0, 1):
                    nc.sync.dma_start(x_sb[:, t, 0:4, :], xT_d[:, t, 0:4, :])
                    nc.sync.dma_start(x_sb[:, t, 4:8, :], xT_d[:, t, 4:8, :])
                for t in (2, 3):
                    nc.sync.dma_start(x_sb[:, t, :, :], xT_d[:, t, :, :])
                nc.scalar.dma_start(wk_sb[:], wk_d[:])
                nc.scalar.dma_start(wv_sb[:], wv_d[:])
                nc.scalar.dma_start(wq_sb[:], wq_d[:])
                nc.scalar.dma_start(cos_sb[:], cos_d[:])
                nc.scalar.dma_start(sin_sb[:], sin_d[:])
                nc.scalar.dma_start(wo_sb[:], wo_d[:])

            # ---------------- building blocks ----------------
            def rope_chunk(dst, ps_tile, t, dbg_cap=False):
                """psum [128, TC] -> dst[:, t*TC:(t+1)*TC] bf16 with RoPE."""
                raw = rope_t.tile([P, TC], BF16, name="raw")
                shuf = rope_t.tile([P, TC], BF16, name="shuf")
                t1 = rope_t.tile([P, TC], BF16, name="t1")
                t2 = rope_t.tile([P, TC], BF16, name="t2")
                nc.vector.tensor_copy(raw[:], ps_tile[:])  # f32 psum -> bf16
                # swap 32-partition halves within each 64-row head block
                for a, b in ((0, 32), (32, 0), (64, 96), (96, 64)):
                    nc.gpsimd.dma_start(shuf[a : a + 32, :], raw[b : b + 32, :])
                cs = cos_sb[:, ts(t, TC)]
                sn = sin_sb[:, ts(t, TC)]
                nc.vector.tensor_mul(t1[:], raw[:], cs)
                nc.vector.tensor_mul(t2[:], shuf[:], sn)
                nc.vector.tensor_add(dst[:, ts(t, TC)], t1[:], t2[:])
                if dbg_cap:
                    nc.gpsimd.dma_start(dbg_ch[:, 1, :], raw[:])
                    nc.gpsimd.dma_start(dbg_ch[:, 2, :], shuf[:])
                    nc.gpsimd.dma_start(dbg_ch[:, 3, :], t1[:])
                    nc.gpsimd.dma_start(dbg_ch[:, 4, :], t2[:])

            def qk_proj_chunk(w_sb, pair, t, dst, ps_qk, dbg_cap=False):
                """One [128, TC] projection chunk + rope. 8 PE matmuls."""
                qp = ps_qk.tile([P, TC], F32, name="qp", tag="qp", bufs=1)
                for h in range(HC):
                    nc.tensor.matmul(
                        qp[:], w_sb[:, h, ts(pair, P)], x_sb[:, t, h, :],
                        start=(h == 0), stop=(h == HC - 1),
                    )
                rope_chunk(dst, qp, t, dbg_cap=dbg_cap)

            def v_chunk(tt, ps_qk):
                """One [128 tok, 256] v tile -> v_all slots. 8 PE matmuls."""
                vp = ps_qk.tile([P, TC], F32, name="vp", tag="kp", bufs=1)
                for h in range(HC):
                    nc.tensor.matmul(
                        vp[:, 0:E_LOCAL], x_sb[:, tt // 4, h, ts(tt % 4, P)], wv_sb[:, h, :],
                        start=(h == 0), stop=(h == HC - 1),
                    )
                for pr in range(N_PAIRS):
                    # slot0 = [V_h0 | 1], slot1 = [V_h1 | 1]
                    nc.vector.tensor_copy(
                        v_all[:, tt, pr, :, 0:HD],
                        vp[:, ts(pr, P)].rearrange("p (s c) -> p s c", c=HD),
                    )

            def o_proj_mm(c, fc, ps_qk, tail=False):
                """One [128, TC] output-projection chunk. 2 PE matmuls + store."""
                op = ps_qk.tile(
                    [P, TC], F32, name="op", tag=("qp" if fc % 2 == 0 else "kp"), bufs=1
                )
                for pair in range(N_PAIRS):
                    nc.tensor.matmul(
                        op[:], wo_sb[:, pair, ts(fc, P)], o_sb[pair][:, ts(c, TC)],
                        start=(pair == 0), stop=(pair == N_PAIRS - 1),
                    )
                ob = outst.tile([P, TC], BF16, name="ob")
                if tail and fc % 2 == 1:
                    nc.scalar.copy(ob[:], op[:])   # ACT is idle after last exp
                else:
                    nc.vector.tensor_copy(ob[:], op[:])
                nc.sync.dma_start(out_d[ts(fc, P), ts(c, TC)], ob[:])

            # ---------------- attention pipeline pieces ----------------
            def scores_exp(pair, c, kt, ps_st):
                st = ps_st.tile([P, 2 * TC], F32, name="st", tag="st")
                nc.tensor.matmul(
                    st[:, 0:TC],
                    k_ro[pair][0:HD, ts(kt, P)],
                    q_ro[pair][0:HD, ts(c, TC)],
                    start=True, stop=True,
                )
                nc.tensor.matmul(
                    st[:, TC : 2 * TC],
                    k_ro[pair][HD:P, ts(kt, P)],
                    q_ro[pair][HD:P, ts(c, TC)],
                    start=True, stop=True,
                    tile_position=(64, 0),
                )
                ex = expp.tile([P, 2 * TC], BF16, name="ex")
                nc.scalar.activation(ex[:], st[:], AF.Exp, scale=SCALE)
                return ex

            def pv_step(pair, c, kt, ex, ots):
                nc.tensor.matmul(
                    ots[0][:], v_all[:, kt, pair, 0, :], ex[:, 0:TC],
                    start=(kt == 0), stop=(kt == N_TT - 1),
                )
                nc.tensor.matmul(
                    ots[1][:], v_all[:, kt, pair, 1, :], ex[:, TC : 2 * TC],
                    start=(kt == 0), stop=(kt == N_TT - 1),
                )

            def norm_c(pair, c, ots):
                # sums are replicated on the "ones" side (psum rows HD:P).
                # Partition moves only ever go DOWN via tensor_copy (up-shifts
                # are silently broken); head1's result is lifted to partitions
                # HD:P by a DMA like baseline.
                ot0, ot1 = ots
                s0 = nrm.tile([HD, TC], F32, name="s0")
                r0 = nrm.tile([HD, TC], F32, name="r0")
                nc.vector.tensor_copy(s0[:], ot0[HD:P, :])
                nc.vector.reciprocal_approx_fast(r0[:], s0[:])
                nc.vector.tensor_mul(o_sb[pair][0:HD, ts(c, TC)], ot0[0:HD, :], r0[:])
                s1 = nrm.tile([HD, TC], F32, name="s1")
                r1 = nrm.tile([HD, TC], F32, name="r1")
                onrm = nrm.tile([HD, TC], BF16, name="onrm")
                nc.vector.tensor_copy(s1[:], ot1[HD:P, :])
                nc.vector.reciprocal_approx_fast(r1[:], s1[:])
                nc.vector.tensor_mul(onrm[:], ot1[0:HD, :], r1[:])
                nc.gpsimd.dma_start(o_sb[pair][HD:P, ts(c, TC)], onrm[:])

            # ---------------- emission schedule ----------------
            # Flat software pipeline over steps s=(pair,c,kt): scores+exp run
            # AHEAD steps in front of PV so the ACT engine never drains at c
            # boundaries. Projection and o_proj work is interleaved into the
            # PE stream at fixed steps (all emission-order deps respected).
            AHEAD = 4
            steps = [
                (pair, c, kt)
                for pair in range(N_PAIRS)
                for c in range(N_TC)
                for kt in range(N_TT)
            ]
            N_STEPS = len(steps)  # 128

            with tc.tile_pool(name="ps_qk", bufs=1, space="PSUM") as ps_qk:
                with tc.tile_pool(name="ps_st", bufs=2, space="PSUM") as ps_st:
                    with tc.tile_pool(name="ps_ot", bufs=1, space="PSUM") as ps_ot:
                        load_inputs()

                        def _qk(w, pair, t, dst, dbg_cap=False):
                            return lambda: qk_proj_chunk(
                                w, pair, t, dst, ps_qk, dbg_cap=dbg_cap and debug
                            )

                        # interleaved PE work, keyed by pipeline loop index i.
                        # Constraints (unit done before): q0[t] < S(p0,c=t);
                        # k1[t] < S(p1,c0,kt=4t); q1[t] < S(p1,c=t); o_proj(c)
                        # after norm(p1,c) which lands at i = 80+16c+1.
                        il = {
                            4: [_qk(wq_sb, 0, 1, q_ro[0])],
                            12: [_qk(wq_sb, 0, 2, q_ro[0])],
                            20: [_qk(wq_sb, 0, 3, q_ro[0])],
                            28: [_qk(wk_sb, 1, 0, k_ro[1])],
                            36: [_qk(wk_sb, 1, 1, k_ro[1])],
                            44: [_qk(wk_sb, 1, 2, k_ro[1])],
                            50: [_qk(wk_sb, 1, 3, k_ro[1])],
                            56: [_qk(wq_sb, 1, 0, q_ro[1], dbg_cap=True)],
                            62: [_qk(wq_sb, 1, 1, q_ro[1])],
                            70: [_qk(wq_sb, 1, 2, q_ro[1])],
                            78: [_qk(wq_sb, 1, 3, q_ro[1])],
                        }
                        for j, fc in enumerate(range(HC)):      # o_proj(c0)
                            il.setdefault(84 + 2 * j, []).append(
                                lambda fc=fc: o_proj_mm(0, fc, ps_qk)
                            )
                        for j, fc in enumerate(range(HC)):      # o_proj(c1)
                            il.setdefault(100 + 2 * j, []).append(
                                lambda fc=fc: o_proj_mm(1, fc, ps_qk)
                            )
                        for j, fc in enumerate(range(HC)):      # o_proj(c2)
                            il.setdefault(115 + 2 * j, []).append(
                                lambda fc=fc: o_proj_mm(2, fc, ps_qk)
                            )

                        ex_q = {}
                        ots = {}

                        def ensure_ots(pair, c):
                            if (pair, c) not in ots:
                                ots[(pair, c)] = (
                                    ps_ot.tile([P, TC], F32, name="ot0", tag="ot0", bufs=1),
                                    ps_ot.tile([P, TC], F32, name="ot1", tag="ot1", bufs=1),
                                )
                            return ots[(pair, c)]

                        for i in range(N_STEPS + AHEAD):
                            # pre-phase work rides in front of its first use
                            if i == 0:
                                qk_proj_chunk(wk_sb, 0, 0, k_ro[0], ps_qk)
                                qk_proj_chunk(wq_sb, 0, 0, q_ro[0], ps_qk)
                                for tt in range(4):
                                    v_chunk(tt, ps_qk)
                            elif i in (1, 2, 3):
                                t = i
                                qk_proj_chunk(wk_sb, 0, t, k_ro[0], ps_qk)
                                for tt in range(4 * t, 4 * t + 4):
                                    v_chunk(tt, ps_qk)

                            if i < N_STEPS:
                                pair, c, kt = steps[i]
                                ensure_ots(pair, c)
                                ex_q[i] = scores_exp(pair, c, kt, ps_st)
                            j = i - AHEAD
                            if j >= 0:
                                pair, c, kt = steps[j]
                                pv_step(pair, c, kt, ex_q.pop(j), ots[(pair, c)])
                                if kt == N_TT - 1:
                                    norm_c(pair, c, ots.pop((pair, c)))
                            for unit in il.get(i, []):
                                unit()

                        for fc in range(HC):
                            o_proj_mm(N_TC - 1, fc, ps_qk, tail=True)

                        if debug:
                            for pr in range(N_PAIRS):
                                nc.gpsimd.dma_start(dbg_q[:, pr, :], q_ro[pr][:])
                                nc.gpsimd.dma_start(dbg_k[:, pr, :], k_ro[pr][:])
                                nc.gpsimd.dma_start(dbg_o[:, pr, :], o_sb[pr][:])
                            nc.sync.dma_start(dbg_v[:], v_all[:].bitcast(F32))

    nc.compile()
    return nc, names


_CACHE = {}


def _get_program():
    if "prog" not in _CACHE:
        _CACHE["prog"] = build_program()
    return _CACHE["prog"]


def _rope_tables():
    inv_freq = 1.0 / (BASE ** (np.arange(0, HD, 2, dtype=np.float64) / HD))
    t = np.arange(L, dtype=np.float64)
    freqs = np.outer(t, inv_freq)            # [L, 32]
    emb = np.concatenate((freqs, freqs), -1)  # [L, 64]
    cos = np.cos(emb).T.astype(np.float32)    # [64, L]
    sin = np.sin(emb).T.astype(np.float32)    # [64, L]
    sin_signed = sin.copy()
    sin_signed[: HD // 2] *= -1.0             # rotate_half sign baked in
    cosT = np.ascontiguousarray(np.concatenate([cos, cos], 0))      # [128, L]
    sinT = np.ascontiguousarray(np.concatenate([sin_signed, sin_signed], 0))
    return cosT, sinT


def _bf16(a):
    import ml_dtypes
    return np.ascontiguousarray(a).astype(ml_dtypes.bfloat16)


def make_in_maps(names, x, Wq, Wk, Wv, Wo):
    cosT, sinT = _rope_tables()
    cosT, sinT = _bf16(cosT), _bf16(sinT)
    in_maps = []
    # x: [1024, 2048] -> [128 p, 4 t, 8 c, 512] so each partition row is one
    # 8KB-contiguous DMA descriptor. Weights similarly partition-major.
    xTs = [
        _bf16(x[b].T.reshape(8, 128, 4, 512).transpose(1, 2, 0, 3))
        for b in range(B)
    ]
    def _w(a):                     # [c*128, e] -> [128, c, e]
        return _bf16(a.reshape(a.shape[0] // 128, 128, -1).transpose(1, 0, 2))
    for core in range(8):
        b = core // 4
        g = core % 4
        es = slice(g * E_LOCAL, (g + 1) * E_LOCAL)
        m = {
            names["in"][0]: xTs[b],
            names["in"][1]: _w(Wq[es, :].T),
            names["in"][2]: _w(Wk[es, :].T),
            names["in"][3]: _w(Wv[es, :].T),
            names["in"][4]: _w(Wo[:, es].T),   # [256,1024] -> [128, 2, 1024]
            names["in"][5]: cosT,
            names["in"][6]: sinT,
        }
        in_maps.append(m)
    return in_maps


def gather_out(names, res):
    out = np.zeros((B, L, HIDDEN), dtype=np.float32)
    for b in range(B):
        acc = np.zeros((HIDDEN, L), dtype=np.float32)
        for g in range(4):
            acc += np.asarray(res.results[b * 4 + g][names["out"]], dtype=np.float32)
        out[b] = acc.T
    return out


def kernel(x, Wq, Wk, Wv, Wo):
    x = np.asarray(x, dtype=np.float32)
    Wq = np.asarray(Wq, dtype=np.float32)
    Wk = np.asarray(Wk, dtype=np.float32)
    Wv = np.asarray(Wv, dtype=np.float32)
    Wo = np.asarray(Wo, dtype=np.float32)

    nc, names = _get_program()
    in_maps = make_in_maps(names, x, Wq, Wk, Wv, Wo)
    res = run_bass_kernel_spmd(nc, in_maps, core_ids=list(range(8)))
    return gather_out(names, res)

